# revision 31
# baseline (speedup 1.0000x reference)
"""Trainium2 Bass kernel for nn_InvestigationBlock (dense transformer block).

Block: LN1 -> qkv -> polynomial-softmax attention -> proj -> +residual
       -> LN2 -> fc1 -> PolyGELU -> fc2 -> +residual

Sharding (8 cores, no collectives): core c handles batch b=c//2 and
query-token half s=c%2 (1024 of 2048 tokens). Each core computes k/v for
the full 2048 tokens of its batch element, everything else only for its
1024 query rows. Output rows are exact and disjoint across cores.

v1 structure (vs v0 baseline):
 - LN1 fused with qkv per 512-token group; h^T built with one 3D-output
   DMA transpose per token tile; q-half LN reuses hkvT (no separate hqT).
 - Bias vectors passed host-pretransposed [128, C] (contiguous DMA).
 - Weights DMA'd on the scalar-engine HWDGE queue; activations/x on sync.
 - fc1/fc2 weights prefetched during attention (after qkv weights die).
 - Scores: head pairs (2g, 2g+1) issued back-to-back as K=64 row-tiled
   matmuls at partition bases 0/64 -> concurrent execution on the PE.
 - Score poly split between ACT path (Square) and DVE path (affine+mul)
   to balance engine load; clamp fused as 2-op tensor_scalar.
 - Normalize: row-sums ride in v's ones-column; reciprocal batched as
   [8, 256] per head pair; 1/r broadcast to 64 partitions via tiny
   masked matmuls (no gpsimd, no [1,512] reciprocals).
"""

import os
import sys

for _p in ("/opt/trn_rl_repo", os.path.expanduser("~/.axon_site/_ro/trn_rl_repo")):
    if os.path.isdir(_p) and _p not in sys.path:
        sys.path.insert(0, _p)

import math
from contextlib import ExitStack

import ml_dtypes
import numpy as np

import concourse.bass as bass
import concourse.mybir as mybir
import concourse.tile as tile
from concourse import bacc
from concourse.bass_utils import run_bass_kernel_spmd
from concourse.masks import make_identity

F32 = mybir.dt.float32
BF16 = mybir.dt.bfloat16
F8 = mybir.dt.float8e4
DR = mybir.MatmulPerfMode.DoubleRow
ATT_SCALE = 512.0  # attnT stored as 512*attn to stay in fp8 range

DIM = 768
HEADS = 12
HD = 64
HIDDEN = 4 * DIM
NTOK = 2048
NQ = 1024
NB = 4
SCALE = HD ** -0.5
LN_EPS = 1e-5
P = 128

KC = DIM // P          # 6 contraction chunks for DIM
TC_KV = NTOK // P      # 16 token tiles (kv)
TC_Q = NQ // P         # 8 token tiles (q)
QCH = NQ // 512        # 2 query chunks of 512
MC_H = HIDDEN // P     # 24 feature chunks of hidden
NG_KV = NTOK // 512    # 4 kv token groups of 512
HP = HEADS // 2        # 6 head pairs

# 1 of every DVE_EVERY score kt-iterations takes the DVE path (rest ACT)
DVE_EVERY = 4
AT_BUFS = 20           # SBUF staging slots (full head + next head's lead)


def _f(x):
    return float(np.asarray(x))


class Cfg:
    """Host-folded constants baked into the program."""

    def __init__(self, inputs):
        a, b, c = _f(inputs["attn_a"]), _f(inputs["attn_b"]), _f(inputs["attn_c"])
        ga, gb, gc = _f(inputs["gelu_a"]), _f(inputs["gelu_b"]), _f(inputs["gelu_c"])
        assert a > 0 and ga > 0
        # a*(Sx)^2 + b*(Sx) + c = (sa*S*x + b/(2sa))^2 + (c - b^2/(4a))
        sa = math.sqrt(a)
        self.attn_scale = sa * SCALE
        self.attn_bias = b / (2 * sa)
        self.attn_d = c - b * b / (4 * a)
        sg = math.sqrt(ga)
        self.gelu_scale = sg
        self.gelu_bias0 = gb / (2 * sg)
        self.gelu_d = gc - gb * gb / (4 * ga)


def build_nc(cfg, v_bias_nonzero, qk_bias_nonzero, pb_nonzero, f2b_nonzero):
    nc = bacc.Bacc(None, target_bir_lowering=False)

    x_kv = nc.dram_tensor("x_kv", [NTOK, DIM], F32, kind="ExternalInput").ap()
    w_qkv = nc.dram_tensor("w_qkv", [DIM, 3 * DIM], F8, kind="ExternalInput").ap()
    w_proj = nc.dram_tensor("w_proj", [DIM, DIM], F8, kind="ExternalInput").ap()
    w_fc1 = nc.dram_tensor("w_fc1", [DIM, HIDDEN], BF16, kind="ExternalInput").ap()
    w_fc2 = nc.dram_tensor("w_fc2", [HIDDEN, DIM], BF16, kind="ExternalInput").ap()
    # per-out-feature bias vectors (fp32), host-pretransposed to [128, C]
    b_qk = nc.dram_tensor("b_qk", [P, 2 * KC], F32, kind="ExternalInput").ap()
    b_v = nc.dram_tensor("b_v", [DIM], F32, kind="ExternalInput").ap()
    b_proj = nc.dram_tensor("b_proj", [P, KC], F32, kind="ExternalInput").ap()
    b_fc2 = nc.dram_tensor("b_fc2", [P, KC], F32, kind="ExternalInput").ap()
    b_gelu = nc.dram_tensor("b_gelu", [P, MC_H], F32, kind="ExternalInput").ap()
    y = nc.dram_tensor("y", [NQ, DIM], F32, kind="ExternalOutput").ap()

    # host reorders x_kv so the q half is always token tiles [0, TC_Q);
    # attention sums over key tokens are permutation-invariant.
    q_t0 = 0

    with tile.TileContext(nc) as tc, ExitStack() as ctx:
        singles = ctx.enter_context(tc.tile_pool(name="singles", bufs=1))

        ident = singles.tile([P, P], F32)
        make_identity(nc, ident)

        eps_sb = singles.tile([P, 1], F32)
        nc.vector.memset(eps_sb, LN_EPS)
        ab_sb = singles.tile([P, 1], F32)
        nc.vector.memset(ab_sb, cfg.attn_bias)
        # mask4[32k, k*64:(k+1)*64] = 1 -> K=128 matmul broadcasts row 32k
        # of the reciprocal staging tile to 64 output partitions
        mask4 = singles.tile([P, 4 * HD], BF16)
        nc.vector.memset(mask4, 0.0)
        for k in range(4):
            nc.vector.memset(mask4[32 * k:32 * k + 1, k * HD:(k + 1) * HD], 1.0)
        rtmp = singles.tile([P, 512], F32)
        nc.vector.memset(rtmp, 1.0)

        b_qk_sb = singles.tile([P, 2 * KC], F32)
        nc.scalar.dma_start(b_qk_sb, b_qk)
        b_proj_sb = singles.tile([P, KC], F32)
        nc.scalar.dma_start(b_proj_sb, b_proj)
        b_fc2_sb = singles.tile([P, KC], F32)
        nc.scalar.dma_start(b_fc2_sb, b_fc2)
        b_gelu_sb = singles.tile([P, MC_H], F32)
        nc.scalar.dma_start(b_gelu_sb, b_gelu)
        if v_bias_nonzero:
            bv_row = singles.tile([1, DIM], F32)
            nc.scalar.dma_start(bv_row, b_v[None, :])
            bv_b = singles.tile([P, DIM], F32)
            nc.gpsimd.partition_broadcast(bv_b, bv_row)

        # residual stream tiles (fp32 token-major); q half loaded as one
        # batched DMA on the gpsimd queue (cheap issue, off the hot queues)
        xq_all = singles.tile([P, TC_Q, DIM], F32, name="xq_all")
        for ch in range(4):
            t0 = ch * 2
            nc.gpsimd.dma_start(
                xq_all[:, t0:t0 + 2, :],
                x_kv[t0 * P:(t0 + 2) * P, :].rearrange("(t p) f -> p t f", p=P))
        xq_tiles = [xq_all[:, t, :] for t in range(TC_Q)]
        x2_tiles = xq_tiles

        # fc1 weights: slot reserved up front (outer pool), DMA issued after
        # the qkv weights die so the load overlaps attention
        poolW = ctx.enter_context(tc.tile_pool(name="poolW", bufs=1))
        wfc1_sb = poolW.tile([P, KC, HIDDEN], BF16, name="wfc1")

        # ---------- pool A2: lives through attention + proj ----------
        ctxA2 = ExitStack()
        poolA2 = ctxA2.enter_context(tc.tile_pool(name="poolA2", bufs=1))
        qT = poolA2.tile([P, KC, NQ], BF16, name="qT")
        kT = poolA2.tile([P, KC, NTOK], BF16, name="kT")
        v_sb = poolA2.tile([P, TC_KV, HEADS, HD + 1], BF16, name="v_sb")
        nc.vector.memset(v_sb[:, :, :, HD:HD + 1], 1.0)

        # ---------- pool A1: LN1 + qkv only ----------
        ctxA1 = ExitStack()
        poolA1 = ctxA1.enter_context(tc.tile_pool(name="poolA1", bufs=1))
        # fp8 weights packed [p, j, o] per kc-pair chunk (DoubleRow layout);
        # 3 chunk DMAs so the first matmuls only wait on chunk 0
        wqkv_sb = [poolA1.tile([P, 2, 3 * DIM], F8, name=f"wqkv{c2}")
                   for c2 in range(KC // 2)]
        for c2 in range(KC // 2):
            nc.scalar.dma_start(
                wqkv_sb[c2],
                w_qkv[2 * c2 * P:(2 * c2 + 2) * P, :]
                .rearrange("(j p) o -> p j o", p=P))
        h8 = poolA1.tile([P, KC, NTOK], F8, name="h8")

        def ln_tile(pool, src_tile, out_bf):
            """token-major LN: out_bf = (x - mean(x)) * rsqrt(var(x)+eps)."""
            stats = pool.tile([P, 2, 6], F32, tag="stats", name="stats")
            nc.vector.bn_stats(stats[:, 0], src_tile[:, 0:512])
            nc.vector.bn_stats(stats[:, 1], src_tile[:, 512:768])
            mv = pool.tile([P, 2], F32, tag="mv", name="mv")
            nc.vector.bn_aggr(mv, stats)
            rstd = pool.tile([P, 1], F32, tag="rstd", name="rstd")
            nc.scalar.activation(rstd, mv[:, 1:2],
                                 mybir.ActivationFunctionType.Sqrt, bias=eps_sb)
            nc.vector.reciprocal(rstd, rstd)
            nc.vector.tensor_scalar(out_bf, src_tile, mv[:, 0:1], rstd,
                                    mybir.AluOpType.subtract, mybir.AluOpType.mult)

        def evac(dst, src, bias_ap):
            if bias_ap is None:
                nc.scalar.activation(dst, src, mybir.ActivationFunctionType.Copy)
            else:
                nc.scalar.activation(dst, src,
                                     mybir.ActivationFunctionType.Identity,
                                     bias=bias_ap)

        # ---------- LN1 + qkv, interleaved per 512-token group ----------
        with tc.tile_pool(name="ln", bufs=3) as ln_pool, \
             tc.tile_pool(name="qkv_ps", bufs=3, space="PSUM") as qkv_ps:
            for g in range(NG_KV):
                # LN + transpose the 4 token tiles of this group
                hg = ln_pool.tile([P, KC, 512], BF16, tag="hg", bufs=2,
                                  name="hg")
                xg = None
                if not (q_t0 <= g * 4 < q_t0 + TC_Q):
                    xg = ln_pool.tile([P, 4, DIM], F32, tag="xg", bufs=2,
                                      name="xg")
                    for ch in range(2):
                        nc.gpsimd.dma_start(
                            xg[:, 2 * ch:2 * ch + 2, :],
                            x_kv[g * 512 + 2 * ch * P:g * 512 + (2 * ch + 2) * P, :]
                            .rearrange("(t p) f -> p t f", p=P))
                for ti in range(4):
                    t = g * 4 + ti
                    if q_t0 <= t < q_t0 + TC_Q:
                        xt = xq_tiles[t - q_t0]
                    else:
                        xt = xg[:, ti, :]
                    ht = ln_pool.tile([P, DIM], BF16, tag="ht", name="ht")
                    ln_tile(ln_pool, xt, ht)
                    nc.sync.dma_start_transpose(
                        hg[:, :, ti * P:(ti + 1) * P], ht)
                gs = slice(g * 512, (g + 1) * 512)
                nc.scalar.activation(h8[:, :, gs], hg,
                                     mybir.ActivationFunctionType.Copy)
                # k^T for this group's 512 tokens
                for mc in range(KC):
                    pt = qkv_ps.tile([P, 512], F32, tag="mm", name="mm")
                    for c2 in range(KC // 2):
                        nc.tensor.matmul(
                            pt,
                            wqkv_sb[c2][:, :, DIM + mc * P:DIM + (mc + 1) * P],
                            h8[:, 2 * c2:2 * c2 + 2, gs],
                            start=(c2 == 0), stop=(c2 == KC // 2 - 1),
                            perf_mode=DR)
                    bias_ap = b_qk_sb[:, KC + mc:KC + mc + 1] if qk_bias_nonzero else None
                    evac(kT[:, mc, gs], pt, bias_ap)
                # v (token-major, per-head with ones col) for this group
                for ti in range(4):
                    t = g * 4 + ti
                    for half in range(2):
                        ncol = 512 if half == 0 else 256
                        nh = ncol // HD
                        pt = qkv_ps.tile([P, 512], F32, tag="mm", name="pt")[:, :ncol]
                        for c2 in range(KC // 2):
                            nc.tensor.matmul(
                                pt,
                                h8[:, 2 * c2:2 * c2 + 2, t * P:(t + 1) * P],
                                wqkv_sb[c2][:, :, 2 * DIM + half * 512:
                                            2 * DIM + half * 512 + ncol],
                                start=(c2 == 0), stop=(c2 == KC // 2 - 1),
                                perf_mode=DR)
                        h0 = half * 8
                        dst = v_sb[:, t, h0:h0 + nh, 0:HD]
                        src = pt.rearrange("p (h d) -> p h d", d=HD)
                        if v_bias_nonzero:
                            nc.vector.tensor_tensor(
                                dst, src,
                                bv_b[:, half * 512:half * 512 + ncol]
                                .rearrange("p (h d) -> p h d", d=HD),
                                mybir.AluOpType.add)
                        else:
                            nc.scalar.activation(dst, src,
                                                 mybir.ActivationFunctionType.Copy)
                # q^T if this group is in the q half
                if q_t0 * P <= g * 512 < (q_t0 + TC_Q) * P:
                    qs = slice(g * 512 - q_t0 * P, g * 512 - q_t0 * P + 512)
                    for mc in range(KC):
                        pt = qkv_ps.tile([P, 512], F32, tag="mm", name="mm")
                        for c2 in range(KC // 2):
                            nc.tensor.matmul(
                                pt,
                                wqkv_sb[c2][:, :, mc * P:(mc + 1) * P],
                                h8[:, 2 * c2:2 * c2 + 2, gs],
                                start=(c2 == 0), stop=(c2 == KC // 2 - 1),
                                perf_mode=DR)
                        bias_ap = b_qk_sb[:, mc:mc + 1] if qk_bias_nonzero else None
                        evac(qT[:, mc, qs], pt, bias_ap)

        ctxA1.close()  # release hkvT + wqkv
        # prefetch fc1 weights during attention (slot was reserved up front);
        # gpsimd SWDGE queue: off the scalar/sync queues the hot path uses
        nc.gpsimd.dma_start(wfc1_sb, w_fc1.rearrange("(c p) o -> p c o", p=P))

        # ---------------- attention + proj ----------------
        ctxAt = ExitStack()
        poolAt = ctxAt.enter_context(tc.tile_pool(name="poolAt", bufs=1))
        attnT = poolAt.tile([P, KC, NQ], F8, name="attnT")
        wproj_sb = poolAt.tile([P, KC // 2, 2, DIM], F8, name="wproj_sb")
        nc.scalar.dma_start(wproj_sb,
                            w_proj.rearrange("(c j p) o -> p c j o", p=P, j=2))

        with tc.tile_pool(name="at", bufs=AT_BUFS) as at_pool, \
             tc.tile_pool(name="sc_ps", bufs=2, space="PSUM") as sc_ps, \
             tc.tile_pool(name="av_ps", bufs=2, space="PSUM") as av_ps:
            pending_tail = [None]

            def run_tail():
                if pending_tail[0] is not None:
                    pending_tail[0]()
                    pending_tail[0] = None

            for g in range(HP):
                # per head: dense score block (long gapless PE run, keeps
                # the HAM clock warm), then dense A@V block consuming the
                # SBUF-staged score tiles; next head's scores overlap
                av2s = {}
                avss = {}
                for par in range(2):
                    h = 2 * g + par
                    base = par * HD
                    at2s = []
                    av2s[par] = av_ps.tile([HD + 1, NQ], F32, tag="av",
                                           name="av")
                    for kt in range(TC_KV):
                        st2 = sc_ps.tile([P, NQ], F32, tag="st", name="st")
                        for qc in range(QCH):
                            nc.tensor.matmul(
                                st2[:, qc * 512:(qc + 1) * 512],
                                kT[base:base + HD, g, kt * P:(kt + 1) * P],
                                qT[base:base + HD, g, qc * 512:(qc + 1) * 512],
                                start=True, stop=True)
                        at2 = at_pool.tile([P, NQ], BF16, tag="a",
                                           bufs=AT_BUFS, name="a")
                        if kt % DVE_EVERY == DVE_EVERY - 1:
                            u = at_pool.tile([P, NQ], BF16, tag="u", bufs=2,
                                             name="u")
                            nc.vector.tensor_scalar(
                                u, st2, cfg.attn_scale, cfg.attn_bias,
                                mybir.AluOpType.mult, mybir.AluOpType.add)
                            nc.vector.tensor_tensor(at2, u, u,
                                                    mybir.AluOpType.mult)
                        else:
                            nc.scalar.activation(
                                at2, st2, mybir.ActivationFunctionType.Square,
                                bias=ab_sb, scale=cfg.attn_scale)
                        nc.vector.tensor_scalar(at2, at2, cfg.attn_d, 1e-6,
                                                mybir.AluOpType.add,
                                                mybir.AluOpType.max)
                        at2s.append(at2)
                    if par == 0:
                        # previous pair's normalize tail: its reciprocal had
                        # the whole score block to complete, so the broadcast
                        # matmuls don't head-of-line-block the PE queue
                        run_tail()
                    for kt in range(TC_KV):
                        for qc in range(QCH):
                            nc.tensor.matmul(
                                av2s[par][:, qc * 512:(qc + 1) * 512],
                                v_sb[:, kt, h, :],
                                at2s[kt][:, qc * 512:(qc + 1) * 512],
                                start=(kt == 0), stop=(kt == TC_KV - 1))
                    # drain: stage row-sums + unnormalized output to SBUF
                    # (releases the av PSUM bank for the next head)
                    for qh in range(2):
                        row = 32 * (2 * par + qh)
                        nc.scalar.activation(
                            rtmp[row:row + 1, :],
                            av2s[par][HD:HD + 1, qh * 512:(qh + 1) * 512],
                            mybir.ActivationFunctionType.Copy,
                            scale=1.0 / ATT_SCALE)
                    avs = at_pool.tile([HD, NQ], BF16, tag="avs", bufs=4,
                                       name="avs")
                    nc.scalar.activation(avs, av2s[par][0:HD, :],
                                         mybir.ActivationFunctionType.Copy)
                    avss[par] = avs

                def tail(g=g, avss=avss):
                    rinv = at_pool.tile([P, 512], BF16, tag="ri", bufs=2,
                                        name="ri")
                    with nc.allow_low_precision(reason="1/r for attention "
                                                "row normalize"):
                        nc.vector.reciprocal(rinv, rtmp)
                    for par in range(2):
                        base = par * HD
                        rb = sc_ps.tile([HD, NQ], F32, tag="st", name="rb")
                        for qh in range(2):
                            idx = 2 * par + qh
                            nc.tensor.matmul(
                                rb[:, qh * 512:(qh + 1) * 512],
                                mask4[:, idx * HD:(idx + 1) * HD], rinv,
                                start=True, stop=True)
                        nc.vector.tensor_tensor(
                            attnT[base:base + HD, g, :],
                            avss[par], rb, mybir.AluOpType.mult)

                pending_tail[0] = tail
            run_tail()

        # ---------------- proj + residual -> x2 ----------------
        with tc.tile_pool(name="pj", bufs=2) as pj_pool, \
             tc.tile_pool(name="pj_ps", bufs=3, space="PSUM") as pj_ps:
            projT = pj_pool.tile([P, KC, NQ], F32, tag="projT", bufs=1, name="projT")
            for mc in range(KC):
                for qc in range(QCH):
                    pt = pj_ps.tile([P, 512], F32, tag="mm", name="mm")
                    for c2 in range(KC // 2):
                        nc.tensor.matmul(
                            pt, wproj_sb[:, c2, :, mc * P:(mc + 1) * P],
                            attnT[:, 2 * c2:2 * c2 + 2, qc * 512:(qc + 1) * 512],
                            start=(c2 == 0), stop=(c2 == KC // 2 - 1),
                            perf_mode=DR)
                    dst = projT[:, mc, qc * 512:(qc + 1) * 512]
                    if pb_nonzero:
                        nc.scalar.activation(dst, pt,
                                             mybir.ActivationFunctionType.Identity,
                                             bias=b_proj_sb[:, mc:mc + 1],
                                             scale=1.0 / ATT_SCALE)
                    else:
                        nc.scalar.activation(dst, pt,
                                             mybir.ActivationFunctionType.Copy,
                                             scale=1.0 / ATT_SCALE)
            for t in range(TC_Q):
                for mc in range(KC):
                    tp = pj_ps.tile([P, P], F32, tag="tr", name="tr")
                    nc.tensor.transpose(tp, projT[:, mc, t * P:(t + 1) * P], ident)
                    nc.vector.scalar_tensor_tensor(
                        x2_tiles[t][:, mc * P:(mc + 1) * P], tp, 1.0,
                        xq_tiles[t][:, mc * P:(mc + 1) * P],
                        mybir.AluOpType.mult, mybir.AluOpType.add)

        ctxAt.close()  # release attnT/wproj/projT
        ctxA2.close()  # release qT/kT/v_sb

        # ---------------- LN2 -> h2^T (+ fc2 weight prefetch) ----------------
        poolB = ctx.enter_context(tc.tile_pool(name="poolB", bufs=1))
        h2T = poolB.tile([P, KC, NQ], BF16, name="h2T")
        wfc2_sb = poolB.tile([P, MC_H, DIM], BF16, name="wfc2")
        nc.gpsimd.dma_start(wfc2_sb, w_fc2.rearrange("(c p) o -> p c o", p=P))
        with tc.tile_pool(name="ln2", bufs=3) as ln2_pool:
            for t in range(TC_Q):
                ht = ln2_pool.tile([P, DIM], BF16, tag="ht", name="ht")
                ln_tile(ln2_pool, x2_tiles[t], ht)
                nc.sync.dma_start_transpose(h2T[:, :, t * P:(t + 1) * P], ht)

        # ---------------- MLP + residual -> y ----------------
        with tc.tile_pool(name="mlp", bufs=2) as mlp_pool, \
             tc.tile_pool(name="mlp_ps", bufs=3, space="PSUM") as mlp_ps:
            for qc in range(QCH):
                gT = mlp_pool.tile([P, MC_H, 512], BF16, tag="gT", bufs=2, name="gT")
                for mc in range(MC_H):
                    pt = mlp_ps.tile([P, 512], F32, tag="mm", name="mm")
                    for kc in range(KC):
                        nc.tensor.matmul(
                            pt, wfc1_sb[:, kc, mc * P:(mc + 1) * P],
                            h2T[:, kc, qc * 512:(qc + 1) * 512],
                            start=(kc == 0), stop=(kc == KC - 1))
                    # PolyGELU: Square(sg*u + bias_vec) + gelu_d
                    nc.scalar.activation(gT[:, mc], pt,
                                         mybir.ActivationFunctionType.Square,
                                         bias=b_gelu_sb[:, mc:mc + 1],
                                         scale=cfg.gelu_scale)
                    nc.vector.tensor_scalar_add(gT[:, mc], gT[:, mc], cfg.gelu_d)
                f2T = mlp_pool.tile([P, KC, 512], F32, tag="f2T", bufs=2, name="f2T")
                for mc in range(KC):
                    pt = mlp_ps.tile([P, 512], F32, tag="mm", name="mm")
                    for kc in range(MC_H):
                        nc.tensor.matmul(
                            pt, wfc2_sb[:, kc, mc * P:(mc + 1) * P],
                            gT[:, kc, :],
                            start=(kc == 0), stop=(kc == MC_H - 1))
                    evac(f2T[:, mc], pt,
                         b_fc2_sb[:, mc:mc + 1] if f2b_nonzero else None)
                for qt in range(4):
                    t = qc * 4 + qt
                    yt = mlp_pool.tile([P, DIM], F32, tag="yt", bufs=2, name="yt")
                    for mc in range(KC):
                        tp = mlp_ps.tile([P, P], F32, tag="tr", name="tr")
                        nc.tensor.transpose(tp, f2T[:, mc, qt * P:(qt + 1) * P],
                                            ident)
                        nc.vector.scalar_tensor_tensor(
                            yt[:, mc * P:(mc + 1) * P], tp, 1.0,
                            x2_tiles[t][:, mc * P:(mc + 1) * P],
                            mybir.AluOpType.mult, mybir.AluOpType.add)
                    nc.sync.dma_start(y[t * P:(t + 1) * P, :], yt)

    nc.compile()
    return nc


_CACHED = {}


def build_common_and_cfg(ins):
    cfg = Cfg(ins)
    ln1_g, ln1_b = ins["ln1_g"].astype(np.float32), ins["ln1_b"].astype(np.float32)
    ln2_g, ln2_b = ins["ln2_g"].astype(np.float32), ins["ln2_b"].astype(np.float32)
    qkv_w = ins["qkv_w"].astype(np.float32)
    fc1_w = ins["fc1_w"].astype(np.float32)

    qkv_w_eff = ln1_g[:, None] * qkv_w
    qkv_b_eff = ins["qkv_b"].astype(np.float32) + ln1_b @ qkv_w
    fc1_w_eff = ln2_g[:, None] * fc1_w
    fc1_b_eff = ins["fc1_b"].astype(np.float32) + ln2_b @ fc1_w

    b_qk = qkv_b_eff[:2 * DIM]
    b_v = qkv_b_eff[2 * DIM:]
    b_proj = ins["proj_b"].astype(np.float32)
    b_fc2 = ins["fc2_b"].astype(np.float32)
    b_gelu = cfg.gelu_scale * fc1_b_eff + cfg.gelu_bias0

    bf = ml_dtypes.bfloat16
    f8 = ml_dtypes.float8_e4m3fn
    common = {
        "w_qkv": np.ascontiguousarray(qkv_w_eff.astype(f8)),
        "w_proj": np.ascontiguousarray(ins["proj_w"].astype(np.float32).astype(f8)),
        "w_fc1": np.ascontiguousarray(fc1_w_eff.astype(bf)),
        "w_fc2": np.ascontiguousarray(ins["fc2_w"].astype(np.float32).astype(bf)),
        "b_qk": np.ascontiguousarray(b_qk.reshape(2 * KC, P).T),
        "b_v": np.ascontiguousarray(b_v),
        "b_proj": np.ascontiguousarray(b_proj.reshape(KC, P).T),
        "b_fc2": np.ascontiguousarray(b_fc2.reshape(KC, P).T),
        "b_gelu": np.ascontiguousarray(b_gelu.reshape(MC_H, P).T),
    }
    flags = (bool(np.any(b_qk != 0.0)), bool(np.any(b_v != 0.0)),
             bool(np.any(b_proj != 0.0)), bool(np.any(b_fc2 != 0.0)))
    return cfg, common, flags


def build_in_maps(ins):
    cfg, common, flags = build_common_and_cfg(ins)
    x = ins["x"].astype(np.float32)
    in_maps = []
    for c in range(8):
        b, s = c // 2, c % 2
        m = dict(common)
        # q half first, other half after (kv order is irrelevant to attention)
        m["x_kv"] = np.ascontiguousarray(
            np.concatenate([x[b, s * NQ:(s + 1) * NQ],
                            x[b, (1 - s) * NQ:(2 - s) * NQ]]))
        in_maps.append(m)
    return cfg, flags, in_maps


def kernel(**inputs) -> np.ndarray:
    ins = {k: np.asarray(v) for k, v in inputs.items()}
    cfg, flags, in_maps = build_in_maps(ins)
    qk_bias_nonzero, v_bias_nonzero, pb_nonzero, f2b_nonzero = flags

    key = (*flags, cfg.attn_scale, cfg.attn_bias, cfg.attn_d,
           cfg.gelu_scale, cfg.gelu_d)
    if key not in _CACHED:
        _CACHED[key] = build_nc(cfg, v_bias_nonzero, qk_bias_nonzero,
                                pb_nonzero, f2b_nonzero)
    nc = _CACHED[key]

    res = run_bass_kernel_spmd(nc, in_maps, core_ids=list(range(8)))

    out = np.empty((NB, NTOK, DIM), dtype=np.float32)
    for c in range(8):
        b, s = c // 2, c % 2
        out[b, s * NQ:(s + 1) * NQ] = res.results[c]["y"]
    return out


if __name__ == "__main__":
    print("use test.py instead")


# revision 34
# speedup vs baseline: 1.0932x; 1.0932x over previous
"""Trainium2 Bass kernel for nn_InvestigationBlock (dense transformer block).

Block: LN1 -> qkv -> polynomial-softmax attention -> proj -> +residual
       -> LN2 -> fc1 -> PolyGELU -> fc2 -> +residual

Sharding (8 cores, no collectives): core c handles batch b=c//2 and
query-token half s=c%2 (1024 of 2048 tokens). Each core computes k/v for
the full 2048 tokens of its batch element, everything else only for its
1024 query rows. Output rows are exact and disjoint across cores.

v1 structure (vs v0 baseline):
 - LN1 fused with qkv per 512-token group; h^T built with one 3D-output
   DMA transpose per token tile; q-half LN reuses hkvT (no separate hqT).
 - Bias vectors passed host-pretransposed [128, C] (contiguous DMA).
 - Weights DMA'd on the scalar-engine HWDGE queue; activations/x on sync.
 - fc1/fc2 weights prefetched during attention (after qkv weights die).
 - Scores: head pairs (2g, 2g+1) issued back-to-back as K=64 row-tiled
   matmuls at partition bases 0/64 -> concurrent execution on the PE.
 - Score poly split between ACT path (Square) and DVE path (affine+mul)
   to balance engine load; clamp fused as 2-op tensor_scalar.
 - Normalize: row-sums ride in v's ones-column; reciprocal batched as
   [8, 256] per head pair; 1/r broadcast to 64 partitions via tiny
   masked matmuls (no gpsimd, no [1,512] reciprocals).
"""

import os
import sys

for _p in ("/opt/trn_rl_repo", os.path.expanduser("~/.axon_site/_ro/trn_rl_repo")):
    if os.path.isdir(_p) and _p not in sys.path:
        sys.path.insert(0, _p)

import math
from contextlib import ExitStack

import ml_dtypes
import numpy as np

import concourse.bass as bass
import concourse.mybir as mybir
import concourse.tile as tile
from concourse import bacc
from concourse.bass_utils import run_bass_kernel_spmd
from concourse.masks import make_identity

F32 = mybir.dt.float32
BF16 = mybir.dt.bfloat16
F8 = mybir.dt.float8e4
DR = mybir.MatmulPerfMode.DoubleRow
ATT_SCALE = 512.0  # attnT stored as 512*attn to stay in fp8 range

DIM = 768
HEADS = 12
HD = 64
HIDDEN = 4 * DIM
NTOK = 2048
NQ = 1024
NB = 4
SCALE = HD ** -0.5
LN_EPS = 1e-5
P = 128

KC = DIM // P          # 6 contraction chunks for DIM
TC_KV = NTOK // P      # 16 token tiles (kv)
TC_Q = NQ // P         # 8 token tiles (q)
QCH = NQ // 512        # 2 query chunks of 512
MC_H = HIDDEN // P     # 24 feature chunks of hidden
NG_KV = NTOK // 512    # 4 kv token groups of 512
HP = HEADS // 2        # 6 head pairs

# 1 of every DVE_EVERY score kt-iterations takes the DVE path (rest ACT)
DVE_EVERY = 4
AT_LAG = 6             # score matmuls run this many kt ahead of A@V
AT_BUFS = 2 * (AT_LAG + 2)  # SBUF staging slots (2 head streams)


def _f(x):
    return float(np.asarray(x))


class Cfg:
    """Host-folded constants baked into the program."""

    def __init__(self, inputs):
        a, b, c = _f(inputs["attn_a"]), _f(inputs["attn_b"]), _f(inputs["attn_c"])
        ga, gb, gc = _f(inputs["gelu_a"]), _f(inputs["gelu_b"]), _f(inputs["gelu_c"])
        assert a > 0 and ga > 0
        # a*(Sx)^2 + b*(Sx) + c = (sa*S*x + b/(2sa))^2 + (c - b^2/(4a))
        sa = math.sqrt(a)
        self.attn_scale = sa * SCALE
        self.attn_bias = b / (2 * sa)
        self.attn_d = c - b * b / (4 * a)
        sg = math.sqrt(ga)
        self.gelu_scale = sg
        self.gelu_bias0 = gb / (2 * sg)
        self.gelu_d = gc - gb * gb / (4 * ga)


def build_nc(cfg, v_bias_nonzero, qk_bias_nonzero, pb_nonzero, f2b_nonzero):
    nc = bacc.Bacc(None, target_bir_lowering=False)

    x_kv = nc.dram_tensor("x_kv", [NTOK, DIM], F32, kind="ExternalInput").ap()
    w_qkv = nc.dram_tensor("w_qkv", [DIM, 3 * DIM], F8, kind="ExternalInput").ap()
    w_proj = nc.dram_tensor("w_proj", [DIM, DIM], F8, kind="ExternalInput").ap()
    w_fc1 = nc.dram_tensor("w_fc1", [DIM, HIDDEN], BF16, kind="ExternalInput").ap()
    w_fc2 = nc.dram_tensor("w_fc2", [HIDDEN, DIM], BF16, kind="ExternalInput").ap()
    # per-out-feature bias vectors (fp32), host-pretransposed to [128, C]
    b_qk = nc.dram_tensor("b_qk", [P, 2 * KC], F32, kind="ExternalInput").ap()
    b_v = nc.dram_tensor("b_v", [DIM], F32, kind="ExternalInput").ap()
    b_proj = nc.dram_tensor("b_proj", [P, KC], F32, kind="ExternalInput").ap()
    b_fc2 = nc.dram_tensor("b_fc2", [P, KC], F32, kind="ExternalInput").ap()
    b_gelu = nc.dram_tensor("b_gelu", [P, MC_H], F32, kind="ExternalInput").ap()
    y = nc.dram_tensor("y", [NQ, DIM], F32, kind="ExternalOutput").ap()

    # host reorders x_kv so the q half is always token tiles [0, TC_Q);
    # attention sums over key tokens are permutation-invariant.
    q_t0 = 0

    with tile.TileContext(nc) as tc, ExitStack() as ctx:
        singles = ctx.enter_context(tc.tile_pool(name="singles", bufs=1))

        ident = singles.tile([P, P], F32)
        make_identity(nc, ident)

        eps_sb = singles.tile([P, 1], F32)
        nc.vector.memset(eps_sb, LN_EPS)
        ab_sb = singles.tile([P, 1], F32)
        nc.vector.memset(ab_sb, cfg.attn_bias)
        # mask4[32k, k*64:(k+1)*64] = 1 -> K=128 matmul broadcasts row 32k
        # of the reciprocal staging tile to 64 output partitions
        mask4 = singles.tile([P, 4 * HD], BF16)
        nc.vector.memset(mask4, 0.0)
        for k in range(4):
            nc.vector.memset(mask4[32 * k:32 * k + 1, k * HD:(k + 1) * HD], 1.0)
        rtmp = singles.tile([P, 512], F32)
        nc.vector.memset(rtmp, 1.0)

        b_qk_sb = singles.tile([P, 2 * KC], F32)
        nc.scalar.dma_start(b_qk_sb, b_qk)
        b_proj_sb = singles.tile([P, KC], F32)
        nc.scalar.dma_start(b_proj_sb, b_proj)
        b_fc2_sb = singles.tile([P, KC], F32)
        nc.scalar.dma_start(b_fc2_sb, b_fc2)
        b_gelu_sb = singles.tile([P, MC_H], F32)
        nc.scalar.dma_start(b_gelu_sb, b_gelu)
        if v_bias_nonzero:
            bv_row = singles.tile([1, DIM], F32)
            nc.scalar.dma_start(bv_row, b_v[None, :])
            bv_b = singles.tile([P, DIM], F32)
            nc.gpsimd.partition_broadcast(bv_b, bv_row)

        # residual stream tiles (fp32 token-major); q half loaded as one
        # batched DMA on the gpsimd queue (cheap issue, off the hot queues)
        xq_all = singles.tile([P, TC_Q, DIM], F32, name="xq_all")
        for ch in range(4):
            t0 = ch * 2
            nc.gpsimd.dma_start(
                xq_all[:, t0:t0 + 2, :],
                x_kv[t0 * P:(t0 + 2) * P, :].rearrange("(t p) f -> p t f", p=P))
        xq_tiles = [xq_all[:, t, :] for t in range(TC_Q)]
        x2_tiles = xq_tiles

        # fc1 weights: slot reserved up front (outer pool), DMA issued after
        # the qkv weights die so the load overlaps attention
        poolW = ctx.enter_context(tc.tile_pool(name="poolW", bufs=1))
        wfc1_sb = poolW.tile([P, KC, HIDDEN], BF16, name="wfc1")

        # ---------- pool A2: lives through attention + proj ----------
        ctxA2 = ExitStack()
        poolA2 = ctxA2.enter_context(tc.tile_pool(name="poolA2", bufs=1))
        qT = poolA2.tile([P, KC, NQ], BF16, name="qT")
        kT = poolA2.tile([P, KC, NTOK], BF16, name="kT")
        v_sb = poolA2.tile([P, TC_KV, HEADS, HD + 1], BF16, name="v_sb")
        nc.vector.memset(v_sb[:, :, :, HD:HD + 1], 1.0)

        # ---------- pool A1: LN1 + qkv only ----------
        ctxA1 = ExitStack()
        poolA1 = ctxA1.enter_context(tc.tile_pool(name="poolA1", bufs=1))
        # fp8 weights packed [p, j, o] per kc-pair chunk (DoubleRow layout);
        # 3 chunk DMAs so the first matmuls only wait on chunk 0
        wqkv_sb = [poolA1.tile([P, 2, 3 * DIM], F8, name=f"wqkv{c2}")
                   for c2 in range(KC // 2)]
        for c2 in range(KC // 2):
            nc.scalar.dma_start(
                wqkv_sb[c2],
                w_qkv[2 * c2 * P:(2 * c2 + 2) * P, :]
                .rearrange("(j p) o -> p j o", p=P))
        h8 = poolA1.tile([P, KC, NTOK], F8, name="h8")

        def ln_tile(pool, src_tile, out_bf):
            """token-major LN: out_bf = (x - mean(x)) * rsqrt(var(x)+eps)."""
            stats = pool.tile([P, 2, 6], F32, tag="stats", name="stats")
            nc.vector.bn_stats(stats[:, 0], src_tile[:, 0:512])
            nc.vector.bn_stats(stats[:, 1], src_tile[:, 512:768])
            mv = pool.tile([P, 2], F32, tag="mv", name="mv")
            nc.vector.bn_aggr(mv, stats)
            rstd = pool.tile([P, 1], F32, tag="rstd", name="rstd")
            nc.scalar.activation(rstd, mv[:, 1:2],
                                 mybir.ActivationFunctionType.Sqrt, bias=eps_sb)
            nc.vector.reciprocal(rstd, rstd)
            nc.vector.tensor_scalar(out_bf, src_tile, mv[:, 0:1], rstd,
                                    mybir.AluOpType.subtract, mybir.AluOpType.mult)

        def evac(dst, src, bias_ap):
            if bias_ap is None:
                nc.scalar.activation(dst, src, mybir.ActivationFunctionType.Copy)
            else:
                nc.scalar.activation(dst, src,
                                     mybir.ActivationFunctionType.Identity,
                                     bias=bias_ap)

        # ---------- LN1 + qkv, interleaved per 512-token group ----------
        with tc.tile_pool(name="ln", bufs=3) as ln_pool, \
             tc.tile_pool(name="qkv_ps", bufs=3, space="PSUM") as qkv_ps:
            for g in range(NG_KV):
                # LN + transpose the 4 token tiles of this group
                hg = ln_pool.tile([P, KC, 512], BF16, tag="hg", bufs=2,
                                  name="hg")
                xg = None
                if not (q_t0 <= g * 4 < q_t0 + TC_Q):
                    xg = ln_pool.tile([P, 4, DIM], F32, tag="xg", bufs=2,
                                      name="xg")
                    for ch in range(2):
                        nc.gpsimd.dma_start(
                            xg[:, 2 * ch:2 * ch + 2, :],
                            x_kv[g * 512 + 2 * ch * P:g * 512 + (2 * ch + 2) * P, :]
                            .rearrange("(t p) f -> p t f", p=P))
                for ti in range(4):
                    t = g * 4 + ti
                    if q_t0 <= t < q_t0 + TC_Q:
                        xt = xq_tiles[t - q_t0]
                    else:
                        xt = xg[:, ti, :]
                    ht = ln_pool.tile([P, DIM], BF16, tag="ht", name="ht")
                    ln_tile(ln_pool, xt, ht)
                    nc.sync.dma_start_transpose(
                        hg[:, :, ti * P:(ti + 1) * P], ht)
                gs = slice(g * 512, (g + 1) * 512)
                nc.scalar.activation(h8[:, :, gs], hg,
                                     mybir.ActivationFunctionType.Copy)
                # k^T for this group's 512 tokens
                for mc in range(KC):
                    pt = qkv_ps.tile([P, 512], F32, tag="mm", name="mm")
                    for c2 in range(KC // 2):
                        nc.tensor.matmul(
                            pt,
                            wqkv_sb[c2][:, :, DIM + mc * P:DIM + (mc + 1) * P],
                            h8[:, 2 * c2:2 * c2 + 2, gs],
                            start=(c2 == 0), stop=(c2 == KC // 2 - 1),
                            perf_mode=DR)
                    bias_ap = b_qk_sb[:, KC + mc:KC + mc + 1] if qk_bias_nonzero else None
                    evac(kT[:, mc, gs], pt, bias_ap)
                # v (token-major, per-head with ones col) for this group
                for ti in range(4):
                    t = g * 4 + ti
                    for half in range(2):
                        ncol = 512 if half == 0 else 256
                        nh = ncol // HD
                        pt = qkv_ps.tile([P, 512], F32, tag="mm", name="pt")[:, :ncol]
                        for c2 in range(KC // 2):
                            nc.tensor.matmul(
                                pt,
                                h8[:, 2 * c2:2 * c2 + 2, t * P:(t + 1) * P],
                                wqkv_sb[c2][:, :, 2 * DIM + half * 512:
                                            2 * DIM + half * 512 + ncol],
                                start=(c2 == 0), stop=(c2 == KC // 2 - 1),
                                perf_mode=DR)
                        h0 = half * 8
                        dst = v_sb[:, t, h0:h0 + nh, 0:HD]
                        src = pt.rearrange("p (h d) -> p h d", d=HD)
                        if v_bias_nonzero:
                            nc.vector.tensor_tensor(
                                dst, src,
                                bv_b[:, half * 512:half * 512 + ncol]
                                .rearrange("p (h d) -> p h d", d=HD),
                                mybir.AluOpType.add)
                        else:
                            nc.scalar.activation(dst, src,
                                                 mybir.ActivationFunctionType.Copy)
                # q^T if this group is in the q half
                if q_t0 * P <= g * 512 < (q_t0 + TC_Q) * P:
                    qs = slice(g * 512 - q_t0 * P, g * 512 - q_t0 * P + 512)
                    for mc in range(KC):
                        pt = qkv_ps.tile([P, 512], F32, tag="mm", name="mm")
                        for c2 in range(KC // 2):
                            nc.tensor.matmul(
                                pt,
                                wqkv_sb[c2][:, :, mc * P:(mc + 1) * P],
                                h8[:, 2 * c2:2 * c2 + 2, gs],
                                start=(c2 == 0), stop=(c2 == KC // 2 - 1),
                                perf_mode=DR)
                        bias_ap = b_qk_sb[:, mc:mc + 1] if qk_bias_nonzero else None
                        evac(qT[:, mc, qs], pt, bias_ap)

        ctxA1.close()  # release hkvT + wqkv
        # prefetch fc1 weights during attention (slot was reserved up front);
        # gpsimd SWDGE queue: off the scalar/sync queues the hot path uses
        nc.gpsimd.dma_start(wfc1_sb, w_fc1.rearrange("(c p) o -> p c o", p=P))

        # ---------------- attention + proj ----------------
        ctxAt = ExitStack()
        poolAt = ctxAt.enter_context(tc.tile_pool(name="poolAt", bufs=1))
        attnT = poolAt.tile([P, KC, NQ], F8, name="attnT")
        wproj_sb = poolAt.tile([P, KC // 2, 2, DIM], F8, name="wproj_sb")
        nc.scalar.dma_start(wproj_sb,
                            w_proj.rearrange("(c j p) o -> p c j o", p=P, j=2))

        with tc.tile_pool(name="at", bufs=AT_BUFS) as at_pool, \
             tc.tile_pool(name="sc_ps", bufs=2, space="PSUM") as sc_ps, \
             tc.tile_pool(name="av_ps", bufs=2, space="PSUM") as av_ps:
            pending_tail = [None]

            def run_tail():
                if pending_tail[0] is not None:
                    pending_tail[0]()
                    pending_tail[0] = None

            for g in range(HP):
                # two interleaved head streams: each stream fills PE gaps
                # left by the other's poly chain
                at2s = {0: [], 1: []}
                av2s = {}
                for par in range(2):
                    av2s[par] = av_ps.tile([HD + 1, NQ], F32, tag="av",
                                           name="av")

                def score_pair(kt):
                    # strict base-0/64 alternation on consecutive matmuls:
                    # disjoint row groups execute concurrently AND keep the
                    # HAM clock warm (half-array runs alone never do)
                    sts = {}
                    for par in range(2):
                        sts[par] = sc_ps.tile([P, NQ], F32, tag="st", name="st")
                    for qc in range(QCH):
                        for par in range(2):
                            base = par * HD
                            nc.tensor.matmul(
                                sts[par][:, qc * 512:(qc + 1) * 512],
                                kT[base:base + HD, g, kt * P:(kt + 1) * P],
                                qT[base:base + HD, g, qc * 512:(qc + 1) * 512],
                                start=True, stop=True)
                    for par in range(2):
                        at2 = at_pool.tile([P, NQ], BF16, tag="a",
                                           bufs=AT_BUFS, name="a")
                        if (2 * kt + par) % DVE_EVERY == DVE_EVERY - 1:
                            u = at_pool.tile([P, NQ], BF16, tag="u", bufs=2,
                                             name="u")
                            nc.vector.tensor_scalar(
                                u, sts[par], cfg.attn_scale, cfg.attn_bias,
                                mybir.AluOpType.mult, mybir.AluOpType.add)
                            nc.vector.tensor_tensor(at2, u, u,
                                                    mybir.AluOpType.mult)
                        else:
                            nc.scalar.activation(
                                at2, sts[par],
                                mybir.ActivationFunctionType.Square,
                                bias=ab_sb, scale=cfg.attn_scale)
                        nc.vector.tensor_scalar(at2, at2, cfg.attn_d, 1e-6,
                                                mybir.AluOpType.add,
                                                mybir.AluOpType.max)
                        at2s[par].append(at2)

                # software pipeline: scores run AT_LAG kt ahead of A@V
                for kt in range(AT_LAG):
                    score_pair(kt)
                run_tail()
                for kt in range(TC_KV):
                    if kt + AT_LAG < TC_KV:
                        score_pair(kt + AT_LAG)
                    for par in range(2):
                        for qc in range(QCH):
                            nc.tensor.matmul(
                                av2s[par][:, qc * 512:(qc + 1) * 512],
                                v_sb[:, kt, 2 * g + par, :],
                                at2s[par][kt][:, qc * 512:(qc + 1) * 512],
                                start=(kt == 0), stop=(kt == TC_KV - 1))
                # drain: stage row-sums + unnormalized head outputs to SBUF
                avss = {}
                for par in range(2):
                    for qh in range(2):
                        row = 32 * (2 * par + qh)
                        nc.scalar.activation(
                            rtmp[row:row + 1, :],
                            av2s[par][HD:HD + 1, qh * 512:(qh + 1) * 512],
                            mybir.ActivationFunctionType.Copy,
                            scale=1.0 / ATT_SCALE)
                    avs = at_pool.tile([HD, NQ], BF16, tag="avs", bufs=4,
                                       name="avs")
                    nc.scalar.activation(avs, av2s[par][0:HD, :],
                                         mybir.ActivationFunctionType.Copy)
                    avss[par] = avs

                def tail(g=g, avss=avss):
                    rinv = at_pool.tile([P, 512], BF16, tag="ri", bufs=2,
                                        name="ri")
                    with nc.allow_low_precision(reason="1/r for attention "
                                                "row normalize"):
                        nc.vector.reciprocal(rinv, rtmp)
                    for par in range(2):
                        base = par * HD
                        rb = sc_ps.tile([HD, NQ], F32, tag="st", name="rb")
                        for qh in range(2):
                            idx = 2 * par + qh
                            nc.tensor.matmul(
                                rb[:, qh * 512:(qh + 1) * 512],
                                mask4[:, idx * HD:(idx + 1) * HD], rinv,
                                start=True, stop=True)
                        nc.vector.tensor_tensor(
                            attnT[base:base + HD, g, :],
                            avss[par], rb, mybir.AluOpType.mult)

                pending_tail[0] = tail
            run_tail()

        # ---------------- proj + residual -> x2 ----------------
        with tc.tile_pool(name="pj", bufs=2) as pj_pool, \
             tc.tile_pool(name="pj_ps", bufs=3, space="PSUM") as pj_ps:
            projT = pj_pool.tile([P, KC, NQ], F32, tag="projT", bufs=1, name="projT")
            for mc in range(KC):
                for qc in range(QCH):
                    pt = pj_ps.tile([P, 512], F32, tag="mm", name="mm")
                    for c2 in range(KC // 2):
                        nc.tensor.matmul(
                            pt, wproj_sb[:, c2, :, mc * P:(mc + 1) * P],
                            attnT[:, 2 * c2:2 * c2 + 2, qc * 512:(qc + 1) * 512],
                            start=(c2 == 0), stop=(c2 == KC // 2 - 1),
                            perf_mode=DR)
                    dst = projT[:, mc, qc * 512:(qc + 1) * 512]
                    if pb_nonzero:
                        nc.scalar.activation(dst, pt,
                                             mybir.ActivationFunctionType.Identity,
                                             bias=b_proj_sb[:, mc:mc + 1],
                                             scale=1.0 / ATT_SCALE)
                    else:
                        nc.scalar.activation(dst, pt,
                                             mybir.ActivationFunctionType.Copy,
                                             scale=1.0 / ATT_SCALE)
            for t in range(TC_Q):
                for mc in range(KC):
                    tp = pj_ps.tile([P, P], F32, tag="tr", name="tr")
                    nc.tensor.transpose(tp, projT[:, mc, t * P:(t + 1) * P], ident)
                    nc.vector.scalar_tensor_tensor(
                        x2_tiles[t][:, mc * P:(mc + 1) * P], tp, 1.0,
                        xq_tiles[t][:, mc * P:(mc + 1) * P],
                        mybir.AluOpType.mult, mybir.AluOpType.add)

        ctxAt.close()  # release attnT/wproj/projT
        ctxA2.close()  # release qT/kT/v_sb

        # ---------------- LN2 -> h2^T (+ fc2 weight prefetch) ----------------
        poolB = ctx.enter_context(tc.tile_pool(name="poolB", bufs=1))
        h2T = poolB.tile([P, KC, NQ], BF16, name="h2T")
        wfc2_sb = poolB.tile([P, MC_H, DIM], BF16, name="wfc2")
        nc.gpsimd.dma_start(wfc2_sb, w_fc2.rearrange("(c p) o -> p c o", p=P))
        with tc.tile_pool(name="ln2", bufs=3) as ln2_pool:
            for t in range(TC_Q):
                ht = ln2_pool.tile([P, DIM], BF16, tag="ht", name="ht")
                ln_tile(ln2_pool, x2_tiles[t], ht)
                nc.sync.dma_start_transpose(h2T[:, :, t * P:(t + 1) * P], ht)

        # ---------------- MLP + residual -> y ----------------
        with tc.tile_pool(name="mlp", bufs=2) as mlp_pool, \
             tc.tile_pool(name="mlp_ps", bufs=3, space="PSUM") as mlp_ps:
            for qc in range(QCH):
                gT = mlp_pool.tile([P, MC_H, 512], BF16, tag="gT", bufs=2, name="gT")
                for mc in range(MC_H):
                    pt = mlp_ps.tile([P, 512], F32, tag="mm", name="mm")
                    for kc in range(KC):
                        nc.tensor.matmul(
                            pt, wfc1_sb[:, kc, mc * P:(mc + 1) * P],
                            h2T[:, kc, qc * 512:(qc + 1) * 512],
                            start=(kc == 0), stop=(kc == KC - 1))
                    # PolyGELU: Square(sg*u + bias_vec) + gelu_d
                    nc.scalar.activation(gT[:, mc], pt,
                                         mybir.ActivationFunctionType.Square,
                                         bias=b_gelu_sb[:, mc:mc + 1],
                                         scale=cfg.gelu_scale)
                    nc.vector.tensor_scalar_add(gT[:, mc], gT[:, mc], cfg.gelu_d)
                f2T = mlp_pool.tile([P, KC, 512], F32, tag="f2T", bufs=2, name="f2T")
                for mc in range(KC):
                    pt = mlp_ps.tile([P, 512], F32, tag="mm", name="mm")
                    for kc in range(MC_H):
                        nc.tensor.matmul(
                            pt, wfc2_sb[:, kc, mc * P:(mc + 1) * P],
                            gT[:, kc, :],
                            start=(kc == 0), stop=(kc == MC_H - 1))
                    evac(f2T[:, mc], pt,
                         b_fc2_sb[:, mc:mc + 1] if f2b_nonzero else None)
                for qt in range(4):
                    t = qc * 4 + qt
                    yt = mlp_pool.tile([P, DIM], F32, tag="yt", bufs=2, name="yt")
                    for mc in range(KC):
                        tp = mlp_ps.tile([P, P], F32, tag="tr", name="tr")
                        nc.tensor.transpose(tp, f2T[:, mc, qt * P:(qt + 1) * P],
                                            ident)
                        nc.vector.scalar_tensor_tensor(
                            yt[:, mc * P:(mc + 1) * P], tp, 1.0,
                            x2_tiles[t][:, mc * P:(mc + 1) * P],
                            mybir.AluOpType.mult, mybir.AluOpType.add)
                    nc.sync.dma_start(y[t * P:(t + 1) * P, :], yt)

    nc.compile()
    return nc


_CACHED = {}


def build_common_and_cfg(ins):
    cfg = Cfg(ins)
    ln1_g, ln1_b = ins["ln1_g"].astype(np.float32), ins["ln1_b"].astype(np.float32)
    ln2_g, ln2_b = ins["ln2_g"].astype(np.float32), ins["ln2_b"].astype(np.float32)
    qkv_w = ins["qkv_w"].astype(np.float32)
    fc1_w = ins["fc1_w"].astype(np.float32)

    qkv_w_eff = ln1_g[:, None] * qkv_w
    qkv_b_eff = ins["qkv_b"].astype(np.float32) + ln1_b @ qkv_w
    fc1_w_eff = ln2_g[:, None] * fc1_w
    fc1_b_eff = ins["fc1_b"].astype(np.float32) + ln2_b @ fc1_w

    b_qk = qkv_b_eff[:2 * DIM]
    b_v = qkv_b_eff[2 * DIM:]
    b_proj = ins["proj_b"].astype(np.float32)
    b_fc2 = ins["fc2_b"].astype(np.float32)
    b_gelu = cfg.gelu_scale * fc1_b_eff + cfg.gelu_bias0

    bf = ml_dtypes.bfloat16
    f8 = ml_dtypes.float8_e4m3fn
    common = {
        "w_qkv": np.ascontiguousarray(qkv_w_eff.astype(f8)),
        "w_proj": np.ascontiguousarray(ins["proj_w"].astype(np.float32).astype(f8)),
        "w_fc1": np.ascontiguousarray(fc1_w_eff.astype(bf)),
        "w_fc2": np.ascontiguousarray(ins["fc2_w"].astype(np.float32).astype(bf)),
        "b_qk": np.ascontiguousarray(b_qk.reshape(2 * KC, P).T),
        "b_v": np.ascontiguousarray(b_v),
        "b_proj": np.ascontiguousarray(b_proj.reshape(KC, P).T),
        "b_fc2": np.ascontiguousarray(b_fc2.reshape(KC, P).T),
        "b_gelu": np.ascontiguousarray(b_gelu.reshape(MC_H, P).T),
    }
    flags = (bool(np.any(b_qk != 0.0)), bool(np.any(b_v != 0.0)),
             bool(np.any(b_proj != 0.0)), bool(np.any(b_fc2 != 0.0)))
    return cfg, common, flags


def build_in_maps(ins):
    cfg, common, flags = build_common_and_cfg(ins)
    x = ins["x"].astype(np.float32)
    in_maps = []
    for c in range(8):
        b, s = c // 2, c % 2
        m = dict(common)
        # q half first, other half after (kv order is irrelevant to attention)
        m["x_kv"] = np.ascontiguousarray(
            np.concatenate([x[b, s * NQ:(s + 1) * NQ],
                            x[b, (1 - s) * NQ:(2 - s) * NQ]]))
        in_maps.append(m)
    return cfg, flags, in_maps


def kernel(**inputs) -> np.ndarray:
    ins = {k: np.asarray(v) for k, v in inputs.items()}
    cfg, flags, in_maps = build_in_maps(ins)
    qk_bias_nonzero, v_bias_nonzero, pb_nonzero, f2b_nonzero = flags

    key = (*flags, cfg.attn_scale, cfg.attn_bias, cfg.attn_d,
           cfg.gelu_scale, cfg.gelu_d)
    if key not in _CACHED:
        _CACHED[key] = build_nc(cfg, v_bias_nonzero, qk_bias_nonzero,
                                pb_nonzero, f2b_nonzero)
    nc = _CACHED[key]

    res = run_bass_kernel_spmd(nc, in_maps, core_ids=list(range(8)))

    out = np.empty((NB, NTOK, DIM), dtype=np.float32)
    for c in range(8):
        b, s = c // 2, c % 2
        out[b, s * NQ:(s + 1) * NQ] = res.results[c]["y"]
    return out


if __name__ == "__main__":
    print("use test.py instead")
